# revision 15
# baseline (speedup 1.0000x reference)
"""Trainium2 Bass kernel for masked causal multi-head attention.

Problem: B=2, T=2048, C=1024, H=16 heads, D=64. Causal + padding mask.

Sharding (8 cores): core g handles heads {2g, 2g+1} for BOTH batches
(tensor-parallel by heads). Every core runs an identical instruction
stream, so load is balanced regardless of the per-batch valid lengths.
Each core computes its qkv slice, attention for its 2 heads, and a
partial output projection (128-row slice of w_out); host unshard sums
the 8 partials, adds b_out, applies the padding mask.

Length specialization: the valid length L_b of each batch is derived
from the mask m on the host; the program is built (and cached) for
lengths rounded up to 128. Tokens beyond ceil128(L_b) are never
touched on device; the host zero-fills them. Keys between L and Lr are
excluded exactly via the mask machinery (V rows zeroed; the softmax
denominator rides column 64 of the augmented V as sum of m_j).

Per-core dataflow (bf16 matmuls, f32 accumulation):
  x is pre-transposed on the HOST, so xT loads are plain wide DMAs (no
  XBAR transpose issue cost). Scores are computed transposed (S^T,
  keys on partitions), exp'd on the Scalar engine (which does nothing
  else, so exps are never queued behind other work), causal-masked on
  diagonal tiles with one DVE multiply per tile. AV uses the [i,d]
  form: lhsT = P^T tile, rhs = V65, so each i-block accumulates
  [128 tok, 65] in PSUM and the denominator lands per-partition;
  normalization is a per-partition reciprocal (DVE) + scale (GpSimd).
  Normalized blocks are PE-transposed back to channel-major for the
  output projection. PSUM note: a matmul with start=True zeroes its
  ENTIRE bank, so exactly the first AV matmul per ov bank uses start.

  Scheduling: attention is software-pipelined (S^T of tile jt+1 issues
  before AV of tile jt, hiding the exp+mask latency), batches'
  i-chunks are interleaved, and qkv/output-projection units are pumped
  between attention tiles by a PE-vs-Act cycle-accounting balance so
  the TensorEngine stays fed during exp waits.
"""

import math
from collections import deque
from functools import partial

import numpy as np
import ml_dtypes

import concourse.bass as bass  # noqa: F401
import concourse.mybir as mybir
import concourse.tile as tile
from concourse import bacc
from concourse.bass_utils import run_bass_kernel_spmd

P = 128
B = 2
T = 2048
C = 1024
CC = C // P      # 8 contract chunks
NHL = 2          # heads per core
LC = NHL * 64    # 128 local channels
ICW = 512        # i-chunk width
SCALE = 64 ** -0.5

dt32 = mybir.dt.float32
dtb = mybir.dt.bfloat16
MM = mybir.ActivationFunctionType

# rough per-row engine costs (ns) for the interleave pacing
PE_NS = 0.417    # 2.4 GHz
ACT_NS = 0.833   # 1.2 GHz


def build(lr0, lr1):
    lr = (lr0, lr1)
    nt = (lr0 // P, lr1 // P)
    nchunks = tuple(math.ceil(l / ICW) for l in lr)
    boff = (0, lr0)          # token offset of each batch in concat layout
    btile = (0, nt[0])       # tile offset of each batch
    ttot = lr0 + lr1
    nttot = nt[0] + nt[1]

    nc = bacc.Bacc("TRN2", target_bir_lowering=False, debug=False)
    xt_ext = [
        nc.declare_dram_parameter(f"xt{b}", [C, lr[b]], dtb, isOutput=False)
        for b in range(B)
    ]
    wq_ext = nc.declare_dram_parameter("wq", [C, LC], dtb, isOutput=False)
    wk_ext = nc.declare_dram_parameter("wk", [C, LC], dtb, isOutput=False)
    wv_ext = nc.declare_dram_parameter("wv", [C, LC], dtb, isOutput=False)
    wo_ext = nc.declare_dram_parameter("wo", [LC, C], dtb, isOutput=False)
    mt_ext = [
        nc.declare_dram_parameter(f"m{b}t", [P, nt[b]], dt32, isOutput=False)
        for b in range(B)
    ]
    tri_ext = nc.declare_dram_parameter("tri", [P, P], dtb, isOutput=False)
    id_ext = nc.declare_dram_parameter("ident", [P, P], dtb, isOutput=False)
    out_ext = [
        nc.declare_dram_parameter(f"out{b}", [lr[b], C], dtb, isOutput=True)
        for b in range(B)
    ]
    out_r = [
        out_ext[b][:].rearrange("(n p) c -> n p c", p=P) for b in range(B)
    ]
    xt_r = [
        xt_ext[b][:].rearrange("(n p) t -> p n t", p=P) for b in range(B)
    ]

    with tile.TileContext(nc) as tc:
        with (
            tc.tile_pool(name="const", bufs=1) as cpool,
            tc.tile_pool(name="big", bufs=1) as big,
            tc.tile_pool(name="stage", bufs=4) as stage,
            tc.tile_pool(name="psPT", bufs=2, space="PSUM") as psPT,
            tc.tile_pool(name="psOV", bufs=1, space="PSUM") as psOV,
            tc.tile_pool(name="psB", bufs=2, space="PSUM") as psB,
        ):
            # ---------------- constants / weights / x ----------------
            msc = cpool.tile([P, nttot], dt32)
            ident = cpool.tile([P, P], dtb)
            tri2 = cpool.tile([P, NHL, P], dtb)
            wq_sb = big.tile([P, CC, LC], dtb)
            wk_sb = big.tile([P, CC, LC], dtb)
            wv_sb = big.tile([P, CC, LC], dtb)
            wo_sb = big.tile([P, C], dtb)

            xT = big.tile([P, CC, ttot], dtb)
            qT = big.tile([P, ttot], dtb)
            kT = big.tile([P, ttot], dtb)
            aoT = big.tile([P, ttot], dtb)
            v_sb = big.tile([P, nttot, NHL, 65], dtb)

            def chunk_w(b, tch):
                return min(ICW, lr[b] - tch * ICW)

            # wq on the Act HW-DGE queue first (its preamble ends a touch
            # earlier), x chunk 0 on SP: the first qkv unit gates startup.
            nc.scalar.dma_start(
                wq_sb[:], wq_ext[:].rearrange("(n p) f -> p n f", p=P))
            x_loads = [(0, t) for t in range(nchunks[0])]
            x_loads += [(1, t) for t in range(nchunks[1])]
            for b, tch in x_loads:
                t0 = tch * ICW
                w = chunk_w(b, tch)
                nc.sync.dma_start(
                    xT[:, :, boff[b] + t0: boff[b] + t0 + w],
                    xt_r[b][:, :, t0:t0 + w],
                )
            for w_ext, w_sb in ((wk_ext, wk_sb), (wv_ext, wv_sb)):
                nc.scalar.dma_start(
                    w_sb[:], w_ext[:].rearrange("(n p) f -> p n f", p=P))
            for b in range(B):
                nc.scalar.dma_start(
                    msc[:, btile[b]:btile[b] + nt[b]], mt_ext[b][:])
            nc.scalar.dma_start(ident[:], id_ext[:])
            for s in range(NHL):
                nc.scalar.dma_start(tri2[:, s, :], tri_ext[:])
            nc.scalar.dma_start(wo_sb[:], wo_ext[:])

            # column 64 of each V tile = m_j: its accumulated row is the
            # softmax denominator (padded keys excluded exactly).
            for s in range(NHL):
                nc.vector.tensor_copy(v_sb[:, :, s, 64:65], msc[:, :, None])

            # ---------------- pacing state ----------------
            est = {"pe": 0.0, "act": 0.0}
            fill = deque()
            qkv_done = [-1, -1]

            def pump_balance():
                while fill and est["pe"] < est["act"]:
                    fill.popleft()()

            def pump_until_qkv(b, tch):
                while qkv_done[b] < tch:
                    fill.popleft()()

            # ---------------- qkv units ----------------
            # each unit is atomic w.r.t. the psB ring: its psum tile is
            # allocated, filled, and copied back with no other psB
            # allocation in between (ring reuse = program order).
            def qk_unit(b, tch, w_sb, dstT):
                w = chunk_w(b, tch)
                t0 = boff[b] + tch * ICW
                ps = psB.tile([P, ICW], dt32, tag="bps", name="qk_ps")
                for cc in range(CC):
                    nc.tensor.matmul(
                        ps[:, :w], w_sb[:, cc, :], xT[:, cc, t0:t0 + w],
                        start=(cc == 0), stop=(cc == CC - 1),
                    )
                est["pe"] += CC * w * PE_NS + 60
                nc.vector.tensor_copy(dstT[:, t0:t0 + w], ps[:, :w])

            def v_pair(b, tch, pair):
                w = chunk_w(b, tch)
                for o in range(2 * pair, min(2 * pair + 2, w // P)):
                    tt = tch * 4 + o
                    gt = btile[b] + tt
                    t0 = boff[b] + tt * P
                    ps = psB.tile([P, LC], dt32, tag="bps", name="v_ps")
                    for cc in range(CC):
                        nc.tensor.matmul(
                            ps[:], xT[:, cc, t0:t0 + P], wv_sb[:, cc, :],
                            start=(cc == 0), stop=(cc == CC - 1),
                        )
                    est["pe"] += 8 * P * PE_NS + 60
                    # zero padded value rows while copying back (DVE —
                    # GpSimd has no PSUM port)
                    nc.vector.tensor_scalar_mul(
                        v_sb[:, gt, :, 0:64],
                        ps[:].rearrange("p (s d) -> p s d", s=NHL),
                        msc[:, gt:gt + 1],
                    )

            def mark_qkv(b, tch):
                qkv_done[b] = tch

            def push_qkv(b, tch):
                for w_sb, dstT in ((wq_sb, qT), (wk_sb, kT)):
                    fill.append(partial(qk_unit, b, tch, w_sb, dstT))
                npair = math.ceil(chunk_w(b, tch) / P / 2)
                for pr in range(npair):
                    fill.append(partial(v_pair, b, tch, pr))
                fill.append(partial(mark_qkv, b, tch))

            def emit_qkv_now(b, tch):
                push_qkv(b, tch)
                pump_until_qkv(b, tch)

            # ---------------- output projection ----------------
            def op_unit(b, tt):
                t0 = boff[b] + tt * P
                ot = stage.tile([P, C], dtb, tag="ot", name="ot")
                for h in range(2):
                    ps = psB.tile([P, ICW], dt32, tag="bps", name="op_ps")
                    nc.tensor.matmul(
                        ps[:], aoT[:, t0:t0 + P],
                        wo_sb[:, h * ICW:(h + 1) * ICW],
                        start=True, stop=True,
                    )
                    est["pe"] += (ICW + P) * PE_NS + 30
                    nc.vector.tensor_copy(ot[:, h * ICW:(h + 1) * ICW],
                                          ps[:])
                nc.sync.dma_start(out_r[b][tt], ot[:])

            # ---------------- attention ----------------
            def score_tile(b, ic, wc, nblk, jt):
                """S^T + exp + diag mask for one j-tile; returns AV args."""
                i0 = boff[b] + ic * ICW
                r = jt - 4 * ic
                off = max(r, 0) * P
                j0 = boff[b] + jt * P
                pt_ps = psPT.tile([P, NHL, ICW], dt32, tag="pt",
                                  name="pt_ps")
                pt_sb = stage.tile([P, NHL, ICW], dtb, tag="pt_sb",
                                   name="pt_sb")
                for s in range(NHL):
                    nc.tensor.matmul(
                        pt_ps[:, s, off:wc],
                        kT[64 * s:64 * s + 64, j0:j0 + P],
                        qT[64 * s:64 * s + 64, i0 + off:i0 + wc],
                        start=True, stop=True,
                    )
                est["pe"] += 2 * (wc - off) * PE_NS + 60
                nc.scalar.activation(
                    pt_sb[:, :, off:wc], pt_ps[:, :, off:wc],
                    MM.Exp, scale=SCALE,
                )
                est["act"] += 2 * (wc - off) * ACT_NS + 500
                if r >= 0:
                    # causal mask on the diagonal tile: GpSimd (SBUF-only
                    # op) keeps it off the DVE and Act queues
                    nc.gpsimd.tensor_mul(
                        pt_sb[:, :, off:off + P],
                        pt_sb[:, :, off:off + P],
                        tri2[:],
                    )
                return (jt, r, pt_sb)

            def av_tile(b, ic, wc, nblk, njt, ov, args):
                jt, r, pt_sb = args
                for s in range(NHL):
                    for blk in range(max(r, 0), nblk):
                        # start=True zeroes the WHOLE psum bank: only the
                        # very first matmul into each ov bank carries it.
                        nc.tensor.matmul(
                            ov[s][:, blk, :],
                            pt_sb[:, s, blk * P:(blk + 1) * P],
                            v_sb[:, btile[b] + jt, s, :],
                            start=(jt == 0 and blk == max(r, 0)),
                            stop=(jt == njt - 1 and blk == nblk - 1),
                        )
                est["pe"] += 2 * (nblk - max(r, 0)) * P * PE_NS + 30

            def attention(b, ic):
                pump_until_qkv(b, ic)
                wc = min(ICW, lr[b] - ic * ICW)
                nblk = wc // P
                njt = 4 * ic + nblk
                i0 = boff[b] + ic * ICW
                ov = [psOV.tile([P, 4, 65], dt32, tag=f"ov{s}",
                                name=f"ov{s}") for s in range(NHL)]
                # software pipeline: S^T/exp of tile jt issues before the
                # AV of tile jt-1, so the PE never sits on a fresh exp.
                prev = None
                for jt in range(njt):
                    args = score_tile(b, ic, wc, nblk, jt)
                    if prev is not None:
                        av_tile(b, ic, wc, nblk, njt, ov, prev)
                    prev = args
                    pump_balance()
                av_tile(b, ic, wc, nblk, njt, ov, prev)
                # normalize + transpose after ALL accumulation groups of
                # the ov banks are closed.
                for blk in range(nblk):
                    ao_t = stage.tile([P, P], dtb, tag="ao_t", name="ao_t")
                    for s in range(NHL):
                        rec = stage.tile([P, 1], dt32, tag="rec",
                                         name="rec")
                        nc.vector.reciprocal_approx_fast(
                            rec[:], ov[s][:, blk, 64:65])
                        nc.vector.tensor_scalar_mul(
                            ao_t[:, 64 * s:64 * s + 64],
                            ov[s][:, blk, 0:64], rec[:])
                    tp = psB.tile([P, P], dtb, tag="bps", name="tp")
                    nc.tensor.transpose(tp[:], ao_t[:], ident[:])
                    est["pe"] += P * PE_NS + 55
                    nc.vector.tensor_copy(
                        aoT[:, i0 + blk * P:i0 + (blk + 1) * P], tp[:])
                # queue the output projection of this i-chunk as filler
                for blk in range(nblk):
                    fill.append(partial(op_unit, b, 4 * ic + blk))

            # ---------------- main sequence ----------------
            emit_qkv_now(0, 0)
            order = []
            for ic in range(max(nchunks)):
                for b in range(B):
                    if ic < nchunks[b]:
                        order.append((b, ic))
            for b, ic in order:
                if not (b == 0 and ic == 0):
                    push_qkv(b, ic)
            for b, ic in order:
                attention(b, ic)
            while fill:
                fill.popleft()()
    nc.finalize()
    return nc


_CACHE = {}


def _get_nc(lr0, lr1):
    key = (lr0, lr1)
    if key not in _CACHE:
        _CACHE[key] = build(lr0, lr1)
    return _CACHE[key]


def _lengths(m):
    lens = np.asarray(m)[:, :, 0].sum(axis=1).astype(int)
    return tuple(max(P, int(math.ceil(l / P)) * P) for l in lens)


def make_in_maps(x, m, w_qkv, w_out):
    bf = ml_dtypes.bfloat16
    x = np.asarray(x)
    m = np.asarray(m)
    w_qkv = np.asarray(w_qkv)
    w_out = np.asarray(w_out)
    lr = _lengths(m)
    xts = [np.ascontiguousarray(x[b, :lr[b]].astype(bf).T) for b in range(B)]
    mts = [
        np.ascontiguousarray(
            m[b, :lr[b], 0].reshape(lr[b] // P, P).T).astype(np.float32)
        for b in range(B)
    ]
    tri = np.triu(np.ones((P, P))).astype(bf)
    ident = np.eye(P).astype(bf)
    in_maps = []
    for g in range(8):
        c0 = g * LC
        in_maps.append({
            "xt0": xts[0], "xt1": xts[1],
            "m0t": mts[0], "m1t": mts[1],
            "tri": tri, "ident": ident,
            "wq": np.ascontiguousarray(w_qkv[:, c0:c0 + LC]).astype(bf),
            "wk": np.ascontiguousarray(w_qkv[:, C + c0:C + c0 + LC]).astype(bf),
            "wv": np.ascontiguousarray(
                w_qkv[:, 2 * C + c0:2 * C + c0 + LC]).astype(bf),
            "wo": np.ascontiguousarray(w_out[c0:c0 + LC, :]).astype(bf),
        })
    return in_maps


def gather(results, m, b_out, lr):
    m = np.asarray(m)
    out = np.zeros((B, T, C), dtype=np.float32)
    for res in results:
        for b in range(B):
            out[b, :lr[b]] += res[f"out{b}"].astype(np.float32)
    out = (out + np.asarray(b_out)[None, None, :]) * m
    return out.astype(np.float32)


def run(x, m, w_qkv, w_out, b_out, trace=False):
    lr = _lengths(m)
    in_maps = make_in_maps(x, m, w_qkv, w_out)
    nc = _get_nc(*lr)
    res = run_bass_kernel_spmd(nc, in_maps, core_ids=list(range(8)),
                               trace=trace)
    return gather(res.results, m, b_out, lr), res


def kernel(x, m, w_qkv, w_out, b_out):
    return run(x, m, w_qkv, w_out, b_out)[0]
